# revision 2
# baseline (speedup 1.0000x reference)
"""Trainium2 Bass kernel v2 for nn_Decoder (GNN message passing decoder).

Reference computation:
    v1 = z_out + z_self                         # [N, C]
    v2 = z_in + z_self                          # [N, C]
    value = v1[src] * v2[dst]                   # [E, C]
    h = elu(value @ W1 + b1)                    # [E, H]
    out = sigmoid(h @ W2 + b2)                  # [E, 1]

v2 strategy (vs the fp32 baseline):
  - fp16 pair tables psrc=[z_out|z_self], pdst=[z_in|z_self] ([N, 2C],
    512B rows): halves gather DMA bytes and descriptor cost (512B is the
    full-rate descriptor-size boundary).
  - dma_gather(transpose=True): gathered rows land channels-on-partitions,
    edges-on-free-dim, eliminating all PE transposes and the PSUM->SBUF
    copy. mm1 consumes the multiplied tile directly.
  - Group pairing: two 512-edge groups stack into h4[0:64]/h4[64:128] so
    ACT elu ops run on full 128 partitions (halves ACT time per edge).
  - mm2 uses a block-diagonal [2H, 2] W2/2 so each pair yields po[2,512];
    four pairs accumulate into one [8,512] PSUM bank -> one Tanh + one
    DVE scale + one 16KB store per 8 groups.
  - elu(s) = relu(s) + exp(min(s,0)) - 1; the -1 folds into b2.
    sigmoid(x) = 0.5*tanh(x/2) + 0.5 keeps the ACT table set fixed
    (exp/relu/tanh share a set; sigmoid does not).
"""
import sys

if "/opt/trn_rl_repo" not in sys.path:
    sys.path.insert(0, "/opt/trn_rl_repo")

import math

import numpy as np

N, C, E, H = 100000, 128, 600000, 64
M = 8                    # cores
EPC = E // M             # edges per core
NCHUNK = 4               # node-table chunks (rows fit in int16)
CHUNK = N // NCHUNK      # 25000
NB = NCHUNK * NCHUNK     # (src_chunk, dst_chunk) buckets
OPG = 5120               # indices per dma_gather op (40 tiles of 128)
PAIRS_PER_OP = OPG // 1024

_BUILD_CACHE: dict = {}


def _build(nops: int, b2p: float, *, nqueues: int = 1,
           gat_bufs: int = 2, vec_bufs: int = 2, stack_bufs: int = 3,
           psum_bufs: int = 2, psumo_bufs: int = 2, ost_bufs: int = 2,
           adds_on: str = "dve", ms_on: str = "act", scale_on: str = "dve",
           do_gather: int = 1, do_compute: int = 1, queue_mode: str = "perop"):
    from concourse import bacc, mybir
    import concourse.tile as tile

    f32 = mybir.dt.float32
    f16 = mybir.dt.float16
    i16 = mybir.dt.int16
    AF = mybir.ActivationFunctionType
    OP = mybir.AluOpType

    cap = nops * OPG
    ncols = NB * nops * (OPG // 16)
    npairs = NB * nops * PAIRS_PER_OP
    assert npairs % 4 == 0
    ngroups = 2 * npairs

    nc = bacc.Bacc("TRN2", target_bir_lowering=False, num_swdge_queues=nqueues)
    psrc = nc.dram_tensor("psrc", [N, 2 * C], f16, kind="ExternalInput")
    pdst = nc.dram_tensor("pdst", [N, 2 * C], f16, kind="ExternalInput")
    isrc = nc.dram_tensor("isrc", [128, ncols], i16, kind="ExternalInput")
    idst = nc.dram_tensor("idst", [128, ncols], i16, kind="ExternalInput")
    w1 = nc.dram_tensor("w1", [C, H], f16, kind="ExternalInput")
    b1s = nc.dram_tensor("b1s", [2 * H, 1], f32, kind="ExternalInput")
    # block-diagonal [W2/2 | W2/2]: pair groups A (rows 0:64) and B (64:128)
    # contract to po[0]/po[1] in one matmul
    w2 = nc.dram_tensor("w2", [2 * H, 2], f16, kind="ExternalInput")
    out = nc.dram_tensor("out", [ngroups, 512], f32, kind="ExternalOutput")

    with tile.TileContext(nc) as tc:
        with (
            tc.tile_pool(name="const", bufs=1) as constp,
            tc.tile_pool(name="gat", bufs=gat_bufs) as gat,
            tc.tile_pool(name="vec", bufs=vec_bufs) as vec,
            tc.tile_pool(name="stack", bufs=stack_bufs) as stackp,
            tc.tile_pool(name="ostp", bufs=ost_bufs) as ostp,
            tc.tile_pool(name="psum", bufs=psum_bufs, space="PSUM") as psum,
            tc.tile_pool(name="psumo", bufs=psumo_bufs, space="PSUM") as psumo,
        ):
            w1t = constp.tile([C, H], f16)
            nc.sync.dma_start(out=w1t[:], in_=w1[:, :])
            b1t = constp.tile([2 * H, 1], f32)
            nc.sync.dma_start(out=b1t[:], in_=b1s[:, :])
            w2t = constp.tile([2 * H, 2], f16)
            nc.sync.dma_start(out=w2t[:], in_=w2[:, :])
            nb1t = constp.tile([2 * H, 1], f32)
            nc.vector.tensor_scalar_mul(nb1t[:], b1t[:], -1.0)
            b2ht = constp.tile([2, 1], f32)
            nc.vector.memset(b2ht[:], float(b2p) * 0.5)
            isrc_t = constp.tile([128, ncols], i16)
            nc.sync.dma_start(out=isrc_t[:], in_=isrc[:, :])
            idst_t = constp.tile([128, ncols], i16)
            nc.sync.dma_start(out=idst_t[:], in_=idst[:, :])

            pp = 0            # global pair counter
            po8 = ot = None
            for b in range(NB):
                sc, dc = divmod(b, NCHUNK)
                for o in range(nops):
                    col0 = (b * nops + o) * (OPG // 16)
                    cols = OPG // 16
                    if queue_mode == "split":
                        qn = (2 * (b * nops + o)) % nqueues
                        qn2 = (2 * (b * nops + o) + 1) % nqueues
                    else:  # per-op
                        qn = qn2 = (b * nops + o) % nqueues
                    sg = gat.tile([128, 2, OPG], f16, tag="sg")
                    dg = gat.tile([128, 2, OPG], f16, tag="dg")
                    if not do_gather:
                        nc.vector.memset(sg[:, :, 0:16], 0.5)
                        nc.vector.memset(dg[:, :, 0:16], 0.5)
                    else:
                        nc.gpsimd.dma_gather(
                            out_ap=sg[:],
                            in_ap=psrc[sc * CHUNK:(sc + 1) * CHUNK, :],
                            idxs_ap=isrc_t[:, col0:col0 + cols],
                            num_idxs=OPG, num_idxs_reg=OPG, elem_size=2 * C,
                            transpose=True, single_packet=False, queue_num=qn,
                        )
                        nc.gpsimd.dma_gather(
                            out_ap=dg[:],
                            in_ap=pdst[dc * CHUNK:(dc + 1) * CHUNK, :],
                            idxs_ap=idst_t[:, col0:col0 + cols],
                            num_idxs=OPG, num_idxs_reg=OPG, elem_size=2 * C,
                            transpose=True, single_packet=False, queue_num=qn2,
                        )
                    if not do_compute:
                        sink = vec.tile([128, 16], f16, tag="sink")
                        nc.vector.tensor_tensor(
                            out=sink[:], in0=sg[:, 0, 0:16], in1=dg[:, 0, 0:16],
                            op=OP.add)
                        continue
                    if adds_on == "pool":
                        adder, adder2 = nc.gpsimd, nc.gpsimd
                    elif adds_on == "dve":
                        adder, adder2 = nc.vector, nc.vector
                    else:  # split
                        adder, adder2 = nc.gpsimd, nc.vector
                    v1 = vec.tile([128, OPG], f16, tag="v1")
                    adder.tensor_tensor(
                        out=v1[:], in0=sg[:, 0, :], in1=sg[:, 1, :], op=OP.add)
                    v2 = vec.tile([128, OPG], f16, tag="v2")
                    adder2.tensor_tensor(
                        out=v2[:], in0=dg[:, 0, :], in1=dg[:, 1, :], op=OP.add)
                    nc.vector.tensor_tensor(
                        out=v1[:], in0=v1[:], in1=v2[:], op=OP.mult)
                    for p in range(PAIRS_PER_OP):
                        eA = v1[:, (2 * p) * 512:(2 * p + 1) * 512]
                        eB = v1[:, (2 * p + 1) * 512:(2 * p + 2) * 512]
                        h4 = psum.tile([128, 512], f32, tag="h4")
                        nc.tensor.matmul(out=h4[0:H, :], lhsT=w1t[:], rhs=eA,
                                         start=True, stop=True)
                        nc.tensor.matmul(out=h4[H:2 * H, :], lhsT=w1t[:],
                                         rhs=eB, start=True, stop=True)
                        hrel = stackp.tile([128, 512], f16, tag="hrel")
                        nc.scalar.activation(
                            out=hrel[:], in_=h4[:], func=AF.Relu, bias=b1t[:])
                        hexp = stackp.tile([128, 512], f16, tag="hexp")
                        if ms_on == "act":
                            ms = stackp.tile([128, 512], f16, tag="ms")
                            nc.scalar.activation(
                                out=ms[:], in_=h4[:], func=AF.Relu,
                                bias=nb1t[:], scale=-1.0)
                            nc.scalar.activation(
                                out=hexp[:], in_=ms[:], func=AF.Exp,
                                scale=-1.0)
                        else:  # dve: ms = min(s, 0) fused (h4 + b1) min 0
                            ms = stackp.tile([128, 512], f16, tag="ms")
                            nc.vector.tensor_scalar(
                                out=ms[:], in0=h4[:], scalar1=b1t[:],
                                scalar2=0.0, op0=OP.add, op1=OP.min)
                            nc.scalar.activation(
                                out=hexp[:], in_=ms[:], func=AF.Exp)
                        po = psumo.tile([2, 512], f32, tag="po")
                        nc.tensor.matmul(
                            out=po[:], lhsT=w2t[:],
                            rhs=hrel[:], start=True, stop=False)
                        nc.tensor.matmul(
                            out=po[:], lhsT=w2t[:],
                            rhs=hexp[:], start=False, stop=True)
                        ot = ostp.tile([2, 512], f32, tag="ot")
                        nc.scalar.activation(
                            out=ot[:], in_=po[:], func=AF.Tanh,
                            bias=b2ht[:])
                        scaler = nc.vector if scale_on == "dve" \
                            else nc.gpsimd
                        scaler.tensor_scalar(
                            out=ot[:], in0=ot[:], scalar1=0.5,
                            scalar2=0.5, op0=OP.mult, op1=OP.add)
                        g0 = 2 * pp
                        nc.sync.dma_start(out=out[g0:g0 + 2, :],
                                          in_=ot[:])
                        pp += 1
    nc.compile()
    return nc


def _wrap_idxs(arr: np.ndarray, nops_total: int) -> np.ndarray:
    """[nops_total * OPG] int16 -> [128, nops_total * OPG//16], 16-partition
    wrapped per op block, replicated 8x across partition groups."""
    a = arr.reshape(nops_total, OPG // 16, 16)
    w16 = a.transpose(2, 0, 1).reshape(16, nops_total * (OPG // 16))
    return np.ascontiguousarray(np.tile(w16, (8, 1)))


def _prep(edge_index: np.ndarray):
    """Bucket each core's edge slice by (src_chunk, dst_chunk)."""
    src = edge_index[0].astype(np.int64)
    dst = edge_index[1].astype(np.int64)

    per_core = []
    max_bucket = 0
    for c in range(M):
        s = src[c * EPC:(c + 1) * EPC]
        d = dst[c * EPC:(c + 1) * EPC]
        bkt = (s // CHUNK) * NCHUNK + (d // CHUNK)
        order = np.argsort(bkt, kind="stable")
        counts = np.bincount(bkt, minlength=NB).astype(np.int64)
        max_bucket = max(max_bucket, int(counts.max()))
        per_core.append((s, d, order, counts))

    nops = max(1, math.ceil(max_bucket / OPG))
    cap = nops * OPG

    prepped = []
    for c in range(M):
        s, d, order, counts = per_core[c]
        isrc_flat = np.zeros(NB * cap, dtype=np.int16)
        idst_flat = np.zeros(NB * cap, dtype=np.int16)
        flat_pos = np.empty(EPC, dtype=np.int64)
        orig_ids = np.empty(EPC, dtype=np.int64)
        ofs = 0
        w = 0
        for b in range(NB):
            k = int(counts[b])
            sel = order[ofs:ofs + k]
            sc, dc = divmod(b, NCHUNK)
            isrc_flat[b * cap:b * cap + k] = (s[sel] - sc * CHUNK).astype(np.int16)
            idst_flat[b * cap:b * cap + k] = (d[sel] - dc * CHUNK).astype(np.int16)
            flat_pos[w:w + k] = b * cap + np.arange(k)
            orig_ids[w:w + k] = c * EPC + sel
            ofs += k
            w += k
        assert w == EPC
        prepped.append((
            _wrap_idxs(isrc_flat, NB * nops),
            _wrap_idxs(idst_flat, NB * nops),
            flat_pos,
            orig_ids,
        ))
    return nops, prepped


def prepare(z_in, z_out, z_self, edge_index, W1, b1, W2, b2):
    """Host-side prep: fp16 pair tables, bucketed int16 indices, weights."""
    z_in = np.asarray(z_in, dtype=np.float32)
    z_out = np.asarray(z_out, dtype=np.float32)
    z_self = np.asarray(z_self, dtype=np.float32)
    edge_index = np.asarray(edge_index)
    W1 = np.asarray(W1, dtype=np.float32)
    b1 = np.asarray(b1, dtype=np.float32)
    W2 = np.asarray(W2, dtype=np.float32)
    b2 = np.asarray(b2, dtype=np.float32)

    psrc = np.concatenate([z_out, z_self], axis=1).astype(np.float16)
    pdst = np.concatenate([z_in, z_self], axis=1).astype(np.float16)
    b2p = float(b2.reshape(-1)[0] - W2.sum())

    nops, prepped = _prep(edge_index)

    w1m = np.ascontiguousarray(W1.astype(np.float16))        # [C, H] = lhsT
    b1sm = np.ascontiguousarray(
        np.concatenate([b1, b1]).reshape(2 * H, 1).astype(np.float32))
    w2h = (W2.reshape(H, 1) * 0.5).astype(np.float16)
    w2m = np.zeros((2 * H, 2), dtype=np.float16)
    w2m[0:H, 0:1] = w2h
    w2m[H:2 * H, 1:2] = w2h

    in_maps = []
    for c in range(M):
        isrc_c, idst_c, _, _ = prepped[c]
        in_maps.append({
            "psrc": psrc, "pdst": pdst,
            "isrc": isrc_c, "idst": idst_c,
            "w1": w1m, "b1s": b1sm, "w2": w2m,
        })
    return nops, b2p, prepped, in_maps


def _run(z_in, z_out, z_self, edge_index, W1, b1, W2, b2, **spmd_kwargs):
    from concourse.bass_utils import run_bass_kernel_spmd

    nops, b2p, prepped, in_maps = prepare(
        z_in, z_out, z_self, edge_index, W1, b1, W2, b2)

    key = (nops, round(b2p, 10))
    if key not in _BUILD_CACHE:
        _BUILD_CACHE.clear()
        _BUILD_CACHE[key] = _build(nops, b2p)
    nc = _BUILD_CACHE[key]

    res = run_bass_kernel_spmd(nc, in_maps, core_ids=list(range(M)), **spmd_kwargs)

    out_full = np.zeros(E, dtype=np.float32)
    for c in range(M):
        _, _, flat_pos, orig_ids = prepped[c]
        core_flat = res.results[c]["out"].reshape(-1)
        out_full[orig_ids] = core_flat[flat_pos]
    return out_full.reshape(E, 1), res


def kernel(z_in, z_out, z_self, edge_index, W1, b1, W2, b2):
    out, _ = _run(z_in, z_out, z_self, edge_index, W1, b1, W2, b2)
    return out
